# revision 40
# baseline (speedup 1.0000x reference)
"""LocallyConnected2d Trainium2 kernel.

Problem: out[b,o,oh,ow] = sum_{c,ki,kj} x[b,c,oh+ki,ow+kj] * W[o,oh,ow,c,ki,kj] + bias[o,oh,ow]
Shapes: x[32,32,64,64], W[64,62,62,32,3,3], bias[64,62,62] -> out[32,64,62,62], all fp32.

Strategy (8 NeuronCores, sharded over output rows, 8 rows/core padded to 64):
- Per output location: 3 accumulating PE matmuls, K=97 each (chunk q = kernel
  row ki; features j=(kj,c) plus a ones-row at j=96 that carries bias on q=2).
- lhsT (stationary) = x patch columns [97,32b]: x is loaded into SBUF once as
  3 column-shifted replicas on partitions kj*32+c, so every lhsT is a direct
  AP slice (no im2col data movement). Partition 96 = constant 1.0.
- rhs (moving) = per-location weights [97,64o], streamed from HBM in
  half-row strips with a host-side layout [row, j, q, ow, o] that makes each
  DMA fully contiguous per partition.
- PSUM accumulates [32b, 64o] per location, 8 locations per bank; DVE copies
  each group to an SBUF out strip; one contiguous DMA per half-row out.
"""

import numpy as np

import concourse.bass as bass  # noqa: F401
import concourse.mybir as mybir
import concourse.tile as tile
from concourse import bacc
from concourse.bass_utils import run_bass_kernel_spmd

B, C_IN, H, W = 32, 32, 64, 64
C_OUT, OH, OW, KK = 64, 62, 62, 3
N_CORES = 8
ROWS = 8          # padded output rows per core (8*8=64 >= 62)
HALF = 31         # locations per strip (half an output row)
XH = ROWS + 2     # input rows needed per core
KP = 97           # contraction per chunk: 96 features + ones/bias row
F32 = mybir.dt.float32

_NC_CACHE = {}


def _build_nc():
    nc = bacc.Bacc(
        "TRN2",
        target_bir_lowering=False,
        debug=False,
        enable_asserts=False,
        num_devices=N_CORES,
    )
    # x ships host-transposed AND pre-shifted into 3 kj-replicas
    # [kj, c, h, w(62), b] so the whole x3 load is one contiguous DMA
    x_d = nc.dram_tensor("x", [KK, C_IN, XH, OW, B], F32, kind="ExternalInput").ap()
    # w ships pre-split by half-row strip: [row, half, j, q, l, o] so each
    # strip DMA is one fully-contiguous block (97 x 23.8KB descriptors)
    w_d = nc.dram_tensor(
        "w", [ROWS, 2, KP, 3, HALF, C_OUT], F32, kind="ExternalInput"
    ).ap()
    ones_d = nc.dram_tensor("ones", [1, XH * OW * B], F32, kind="ExternalInput").ap()
    # out layout: [row, half, p=(l4,b), grp, o] - 4 locations (col groups)
    # stacked on PSUM/SBUF partitions; host unscrambles
    NG = 8  # ceil(31/4) location groups per strip
    o_d = nc.dram_tensor(
        "out", [ROWS, 2, 128, NG, C_OUT], F32, kind="ExternalOutput"
    ).ap()

    with tile.TileContext(nc) as tc:
        with (
            tc.tile_pool(name="xpool", bufs=1) as xpool,
            tc.tile_pool(name="wpool", bufs=5) as wpool,
            tc.tile_pool(name="opool", bufs=2) as opool,
            tc.tile_pool(name="pspool", bufs=8, space="PSUM") as pspool,
        ):
            # x replicas: partition kj*32+c holds x[b,c,h,w+kj] at free
            # (h, w, b); partition 96 = 1.0 (carries the bias row).
            # Contiguous layout -> large (39.7KB) DMA descriptors; throughput
            # comes from multiple concurrent sub-DMAs (each in-flight
            # InstDMACopy has its own outstanding-descriptor window).
            HZ = OW * B  # 1984
            x3 = xpool.tile([KP, XH * HZ], F32)
            nc.sync.dma_start(out=x3[96:97, :], in_=ones_d)
            xsrc = x_d.rearrange("k c h w b -> (k c) (h w b)")

            def load_x_rows(r0, r1, eng=None):
                for p0, p1 in ((0, 32), (32, 64), (64, 96)):
                    (eng or nc.gpsimd).dma_start(
                        out=x3[p0:p1, r0 * HZ : r1 * HZ],
                        in_=xsrc[p0:p1, r0 * HZ : r1 * HZ],
                    )

            # rows 0-2 up front (first output row); rows 4-7 ride the
            # otherwise-idle sync HWDGE ring (26 GB/s but off the main
            # stream); rows 3, 8, 9 interleave into the strip loop
            load_x_rows(0, 3)
            load_x_rows(4, 8, eng=nc.sync)

            QZ = HALF * C_OUT  # 1984, one chunk per kernel row q
            XROW_PREFETCH = {0: (3, 4), 2: (8, 9), 3: (9, 10)}
            for row in range(ROWS):
                # prefetch x rows needed a few output rows ahead
                if row in XROW_PREFETCH:
                    load_x_rows(*XROW_PREFETCH[row])
                for half in range(2):
                    wt = wpool.tile([KP, 3 * QZ], F32, tag="wt")
                    # 3 sub-DMAs by partition range -> 3 concurrent windows,
                    # each with one 23.8KB contiguous descriptor per partition
                    wsrc = w_d[row, half].rearrange("p q l o -> p (q l o)")
                    for p0, p1 in ((0, 32), (32, 64), (64, KP)):
                        nc.gpsimd.dma_start(out=wt[p0:p1, :], in_=wsrc[p0:p1])
                    ot = opool.tile([128, NG * C_OUT], F32, tag="ot")
                    otv = ot.rearrange("p (g o) -> p g o", g=NG, o=C_OUT)
                    for g in range(NG):
                        gn = min(4, HALF - g * 4)  # 4,4,...,3
                        # 4 locations packed into PE col groups: out slice
                        # base partition 32*l selects the col group, so the
                        # 4 locations' matmuls can overlap in the array
                        ps = pspool.tile([128, C_OUT], F32, tag="ps")
                        for li in range(4):
                            # pad slot in the last group duplicates the prior
                            # location (keeps PSUM fully written; host drops it)
                            eff = min(li, gn - 1)
                            ow = half * HALF + g * 4 + eff
                            for q in range(3):
                                loff = (g * 4 + eff) * C_OUT
                                nc.tensor.matmul(
                                    ps[32 * li : 32 * li + 32, :],
                                    x3[
                                        :,
                                        (row + q) * HZ
                                        + ow * B : (row + q) * HZ
                                        + ow * B
                                        + B,
                                    ],  # [97, 32] lhsT
                                    wt[:, q * QZ + loff : q * QZ + loff + C_OUT],
                                    start=(q == 0),
                                    stop=(q == 2),
                                    tile_position=(0, 32 * li),
                                )
                        nc.vector.tensor_copy(out=otv[:, g, :], in_=ps)
                    # scalar = second HWDGE ring: keeps out-stores off the
                    # gpsimd FIFO so w prefetch is never head-of-line blocked
                    nc.scalar.dma_start(
                        out=o_d[row, half], in_=ot, max_dma_last_dim=992
                    )

    nc.compile()
    return nc


def get_nc():
    if "nc" not in _NC_CACHE:
        _NC_CACHE["nc"] = _build_nc()
    return _NC_CACHE["nc"]


def prep_inputs(x, weight, bias):
    """Host-side shard + layout prep. Returns per-core in_maps."""
    x = np.asarray(x, dtype=np.float32)
    weight = np.asarray(weight, dtype=np.float32)
    bias = np.asarray(bias, dtype=np.float32)

    # w_prep[oh, j=kj*32+c, q=ki, ow, o]; j=96 row: 0 for q<2, bias for q=2
    wp = np.zeros((N_CORES * ROWS, KP, 3, OW, C_OUT), np.float32)
    wp[:OH, :96] = weight.transpose(1, 5, 3, 4, 2, 0).reshape(OH, 96, 3, OW, C_OUT)
    wp[:OH, 96, 2] = bias.transpose(1, 2, 0)
    # split ow into half-row strips: [row, half, j, q, l, o]
    wp = np.ascontiguousarray(
        wp.reshape(N_CORES * ROWS, KP, 3, 2, HALF, C_OUT).transpose(0, 3, 1, 2, 4, 5)
    )

    xp = np.zeros((B, C_IN, N_CORES * ROWS + 2, W), np.float32)
    xp[:, :, :H] = x
    xt = xp.transpose(1, 2, 3, 0)  # [c, h, w, b]

    ones = np.ones((1, XH * OW * B), np.float32)

    in_maps = []
    for c in range(N_CORES):
        r0 = c * ROWS
        xc = xt[:, r0 : r0 + XH]  # [c, 10, 64, b]
        xsh = np.stack([xc[:, :, kj : kj + OW, :] for kj in range(KK)])
        in_maps.append(
            {
                "x": np.ascontiguousarray(xsh),
                "w": np.ascontiguousarray(wp[r0 : r0 + ROWS]),
                "ones": ones,
            }
        )
    return in_maps


def gather_output(results):
    """results: list of per-core out dicts -> full [B, C_OUT, OH, OW]."""
    out = np.empty((B, C_OUT, OH, OW), np.float32)
    for c in range(N_CORES):
        oc = results[c]["out"]  # [ROWS, 2, 128=(l4,b), NG, C_OUT]
        v = oc.reshape(ROWS, 2, 4, B, 8, C_OUT)
        # ow = half*31 + grp*4 + l  (grp*4+l < 31)
        arr = v.transpose(3, 5, 0, 1, 4, 2).reshape(B, C_OUT, ROWS, 2, 32)
        arr = arr[:, :, :, :, :HALF].reshape(B, C_OUT, ROWS, OW)
        r0 = c * ROWS
        rows = min(ROWS, OH - r0)
        out[:, :, r0 : r0 + rows, :] = arr[:, :, :rows, :]
    return out


def run(inputs, **kw):
    nc = get_nc()
    in_maps = prep_inputs(inputs["x"], inputs["weight"], inputs["bias"])
    res = run_bass_kernel_spmd(nc, in_maps, core_ids=list(range(N_CORES)), **kw)
    return gather_output(res.results), res


def kernel(x, weight, bias):
    out, _ = run({"x": x, "weight": weight, "bias": bias})
    return out


# revision 42
# speedup vs baseline: 1.0314x; 1.0314x over previous
"""LocallyConnected2d Trainium2 kernel.

Problem: out[b,o,oh,ow] = sum_{c,ki,kj} x[b,c,oh+ki,ow+kj] * W[o,oh,ow,c,ki,kj] + bias[o,oh,ow]
Shapes: x[32,32,64,64], W[64,62,62,32,3,3], bias[64,62,62] -> out[32,64,62,62], all fp32.

Strategy (8 NeuronCores, sharded over output rows, 8 rows/core padded to 64):
- Per output location: 3 accumulating PE matmuls, K=97 each (chunk q = kernel
  row ki; features j=(kj,c) plus a ones-row at j=96 that carries bias on q=2).
- lhsT (stationary) = x patch columns [97,32b]: x is loaded into SBUF once as
  3 column-shifted replicas on partitions kj*32+c, so every lhsT is a direct
  AP slice (no im2col data movement). Partition 96 = constant 1.0.
- rhs (moving) = per-location weights [97,64o], streamed from HBM in
  half-row strips with a host-side layout [row, j, q, ow, o] that makes each
  DMA fully contiguous per partition.
- PSUM accumulates [32b, 64o] per location, 8 locations per bank; DVE copies
  each group to an SBUF out strip; one contiguous DMA per half-row out.
"""

import numpy as np

import concourse.bass as bass  # noqa: F401
import concourse.mybir as mybir
import concourse.tile as tile
from concourse import bacc
from concourse.bass_utils import run_bass_kernel_spmd

B, C_IN, H, W = 32, 32, 64, 64
C_OUT, OH, OW, KK = 64, 62, 62, 3
N_CORES = 8
ROWS = 8          # padded output rows per core (8*8=64 >= 62)
HALF = 31         # locations per strip (half an output row)
XH = ROWS + 2     # input rows needed per core
KP = 97           # contraction per chunk: 96 features + ones/bias row
F32 = mybir.dt.float32

_NC_CACHE = {}


def _build_nc():
    nc = bacc.Bacc(
        "TRN2",
        target_bir_lowering=False,
        debug=False,
        enable_asserts=False,
        num_devices=N_CORES,
    )
    # x ships host-transposed AND pre-shifted into 3 kj-replicas
    # [kj, c, h, w(62), b] so the whole x3 load is one contiguous DMA
    x_d = nc.dram_tensor("x", [KK, C_IN, XH, OW, B], F32, kind="ExternalInput").ap()
    # w ships pre-split by half-row strip: [row, half, j, q, l, o] so each
    # strip DMA is one fully-contiguous block (97 x 23.8KB descriptors)
    w_d = nc.dram_tensor(
        "w", [ROWS, 2, KP, 3, HALF, C_OUT], F32, kind="ExternalInput"
    ).ap()
    ones_d = nc.dram_tensor("ones", [1, XH * OW * B], F32, kind="ExternalInput").ap()
    # out layout: [row, half, p=(l4,b), grp, o] - 4 locations (col groups)
    # stacked on PSUM/SBUF partitions; host unscrambles
    NG = 8  # ceil(31/4) location groups per strip
    o_d = nc.dram_tensor(
        "out", [ROWS, 2, 128, NG, C_OUT], F32, kind="ExternalOutput"
    ).ap()

    with tile.TileContext(nc) as tc:
        with (
            tc.tile_pool(name="xpool", bufs=1) as xpool,
            tc.tile_pool(name="wpool", bufs=5) as wpool,
            tc.tile_pool(name="opool", bufs=2) as opool,
            tc.tile_pool(name="pspool", bufs=8, space="PSUM") as pspool,
        ):
            # x replicas: partition kj*32+c holds x[b,c,h,w+kj] at free
            # (h, w, b); partition 96 = 1.0 (carries the bias row).
            # Contiguous layout -> large (39.7KB) DMA descriptors; throughput
            # comes from multiple concurrent sub-DMAs (each in-flight
            # InstDMACopy has its own outstanding-descriptor window).
            HZ = OW * B  # 1984
            x3 = xpool.tile([KP, XH * HZ], F32)
            nc.sync.dma_start(out=x3[96:97, :], in_=ones_d)
            xsrc = x_d.rearrange("k c h w b -> (k c) (h w b)")

            def load_x_rows(r0, r1, eng=None):
                for p0, p1 in ((0, 32), (32, 64), (64, 96)):
                    (eng or nc.gpsimd).dma_start(
                        out=x3[p0:p1, r0 * HZ : r1 * HZ],
                        in_=xsrc[p0:p1, r0 * HZ : r1 * HZ],
                    )

            # rows 0-2 up front (first output row), per-row granularity so
            # early matmuls unblock ASAP; rows 4-7 ride the otherwise-idle
            # sync HWDGE ring; rows 3, 8, 9 interleave into the strip loop
            for r in range(3):
                load_x_rows(r, r + 1)
            load_x_rows(4, 8, eng=nc.sync)

            QZ = HALF * C_OUT  # 1984, one chunk per kernel row q
            XROW_PREFETCH = {(0, 1): (3, 4), (2, 0): (8, 9), (3, 0): (9, 10)}
            for row in range(ROWS):
                for half in range(2):
                    if (row, half) in XROW_PREFETCH:
                        load_x_rows(*XROW_PREFETCH[(row, half)])
                    strip = row * 2 + half
                    wt = wpool.tile([KP, 3 * QZ], F32, tag="wt")
                    # 3 sub-DMAs by partition range -> 3 concurrent windows,
                    # each with one 23.8KB contiguous descriptor per partition.
                    # First/last strips split additionally by q-chunk so the
                    # first q=0 matmuls unblock after 1/3 of the strip.
                    wsrc = w_d[row, half].rearrange("p q l o -> p (q l o)")
                    qsplit = ((0, QZ), (QZ, 2 * QZ), (2 * QZ, 3 * QZ))
                    if strip in (0, 15):
                        for f0, f1 in qsplit:
                            for p0, p1 in ((0, 32), (32, 64), (64, KP)):
                                nc.gpsimd.dma_start(
                                    out=wt[p0:p1, f0:f1], in_=wsrc[p0:p1, f0:f1]
                                )
                    else:
                        for p0, p1 in ((0, 32), (32, 64), (64, KP)):
                            nc.gpsimd.dma_start(out=wt[p0:p1, :], in_=wsrc[p0:p1])
                    ot = opool.tile([128, NG * C_OUT], F32, tag="ot")
                    otv = ot.rearrange("p (g o) -> p g o", g=NG, o=C_OUT)
                    for g in range(NG):
                        gn = min(4, HALF - g * 4)  # 4,4,...,3
                        # 4 locations packed into PE col groups: out slice
                        # base partition 32*l selects the col group, so the
                        # 4 locations' matmuls can overlap in the array
                        ps = pspool.tile([128, C_OUT], F32, tag="ps")
                        for li in range(4):
                            # pad slot in the last group duplicates the prior
                            # location (keeps PSUM fully written; host drops it)
                            eff = min(li, gn - 1)
                            ow = half * HALF + g * 4 + eff
                            for q in range(3):
                                loff = (g * 4 + eff) * C_OUT
                                nc.tensor.matmul(
                                    ps[32 * li : 32 * li + 32, :],
                                    x3[
                                        :,
                                        (row + q) * HZ
                                        + ow * B : (row + q) * HZ
                                        + ow * B
                                        + B,
                                    ],  # [97, 32] lhsT
                                    wt[:, q * QZ + loff : q * QZ + loff + C_OUT],
                                    start=(q == 0),
                                    stop=(q == 2),
                                    tile_position=(0, 32 * li),
                                )
                        nc.vector.tensor_copy(out=otv[:, g, :], in_=ps)
                    # scalar = second HWDGE ring: keeps out-stores off the
                    # gpsimd FIFO so w prefetch is never head-of-line blocked
                    nc.scalar.dma_start(
                        out=o_d[row, half], in_=ot, max_dma_last_dim=992
                    )

    nc.compile()
    return nc


def get_nc():
    if "nc" not in _NC_CACHE:
        _NC_CACHE["nc"] = _build_nc()
    return _NC_CACHE["nc"]


def prep_inputs(x, weight, bias):
    """Host-side shard + layout prep. Returns per-core in_maps."""
    x = np.asarray(x, dtype=np.float32)
    weight = np.asarray(weight, dtype=np.float32)
    bias = np.asarray(bias, dtype=np.float32)

    # w_prep[oh, j=kj*32+c, q=ki, ow, o]; j=96 row: 0 for q<2, bias for q=2
    wp = np.zeros((N_CORES * ROWS, KP, 3, OW, C_OUT), np.float32)
    wp[:OH, :96] = weight.transpose(1, 5, 3, 4, 2, 0).reshape(OH, 96, 3, OW, C_OUT)
    wp[:OH, 96, 2] = bias.transpose(1, 2, 0)
    # split ow into half-row strips: [row, half, j, q, l, o]
    wp = np.ascontiguousarray(
        wp.reshape(N_CORES * ROWS, KP, 3, 2, HALF, C_OUT).transpose(0, 3, 1, 2, 4, 5)
    )

    xp = np.zeros((B, C_IN, N_CORES * ROWS + 2, W), np.float32)
    xp[:, :, :H] = x
    xt = xp.transpose(1, 2, 3, 0)  # [c, h, w, b]

    ones = np.ones((1, XH * OW * B), np.float32)

    in_maps = []
    for c in range(N_CORES):
        r0 = c * ROWS
        xc = xt[:, r0 : r0 + XH]  # [c, 10, 64, b]
        xsh = np.stack([xc[:, :, kj : kj + OW, :] for kj in range(KK)])
        in_maps.append(
            {
                "x": np.ascontiguousarray(xsh),
                "w": np.ascontiguousarray(wp[r0 : r0 + ROWS]),
                "ones": ones,
            }
        )
    return in_maps


def gather_output(results):
    """results: list of per-core out dicts -> full [B, C_OUT, OH, OW]."""
    out = np.empty((B, C_OUT, OH, OW), np.float32)
    for c in range(N_CORES):
        oc = results[c]["out"]  # [ROWS, 2, 128=(l4,b), NG, C_OUT]
        v = oc.reshape(ROWS, 2, 4, B, 8, C_OUT)
        # ow = half*31 + grp*4 + l  (grp*4+l < 31)
        arr = v.transpose(3, 5, 0, 1, 4, 2).reshape(B, C_OUT, ROWS, 2, 32)
        arr = arr[:, :, :, :, :HALF].reshape(B, C_OUT, ROWS, OW)
        r0 = c * ROWS
        rows = min(ROWS, OH - r0)
        out[:, :, r0 : r0 + rows, :] = arr[:, :, :rows, :]
    return out


def run(inputs, **kw):
    nc = get_nc()
    in_maps = prep_inputs(inputs["x"], inputs["weight"], inputs["bias"])
    res = run_bass_kernel_spmd(nc, in_maps, core_ids=list(range(N_CORES)), **kw)
    return gather_output(res.results), res


def kernel(x, weight, bias):
    out, _ = run({"x": x, "weight": weight, "bias": bias})
    return out
